# revision 60
# baseline (speedup 1.0000x reference)
"""Transformer-XL attention kernel for 8 TRN2 NeuronCores — fp8 DoubleRow.

Sharding: data-parallel over batch B=4 x 2-way split of query rows
(interleaved 128-row tiles for mask balance). No collectives.

Math restructure vs the bf16 baseline:
  (q+u)k^T + (q+v)r^T  =  q(k+r)^T + [u.k + v.r]     (per head)
The second term is a per-key bias row, produced by a skinny matmul and
folded into the score matmul as an extra contraction row (so the exp
activation needs no per-tile bias operand and can batch 2 tiles).

All matmuls run fp8e4 with perf_mode=DoubleRow (2 contraction rows per
partition, operands shaped [P, 2, N]):
  - projections (k+r / q / v / bias): weights host-packed in DR layout
  - scores: contraction 64+1 (kr dims + bias row); kr/q psum evictions
    are reshuffled into the DR layout by SBUF->SBUF DMAs (adjacent-dim
    pairing (2d, 2d+1) so each reshuffle is one contiguous DMA)
  - ctx: DR pairs adjacent tk tiles (union widths are pair-equal)
  - out-proj: ctx eviction writes the DR layout directly

Scales (host-staged): Wk,Wr,Wq,Wv x32, Wo x64, wbias x256; exp scale
2^-13 recovers true logits/sqrt(dv); vq ones=2.0; out evict 2^-10.

Engine budget (TimelineSim): ACT ~82us (exp, the critical stream, kept
saturated by pipelining scores(h+1) ahead of ctx(h)), DVE ~94us (psum
evictions + masks + normalize + LN; GPSIMD cannot read PSUM), PE ~65us,
Pool ~23us (SBUF-only ops). Head 0 is interleaved with the kr/bias tk
chunks it consumes so the first exp fires as soon as chunk 0 lands.
"""

import numpy as np
import ml_dtypes

import concourse.bass as bass
from concourse import bacc
import concourse.mybir as mybir
import concourse.tile as tile
from concourse.bass_utils import run_bass_kernel_spmd

B, TQ, TK, D, H, DV = 4, 1024, 1536, 1024, 16, 64
NTK = 12
QSLOTS = {0: [0, 3, 4, 7], 1: [1, 2, 5, 6]}
# union structural widths per tk tile (512 - 128*fp)
FP_UNION = [0, 0, 0, 0, 0, 0, 1, 1, 2, 2, 3, 3]
MASK_POS = [(4, 0), (5, 0), (6, 1), (7, 1), (8, 2), (9, 2), (10, 3), (11, 3)]
# pair widths for tk-tile pairs (0,1),(2,3),...,(10,11) — equal within pair
PAIR_W = [512, 512, 512, 384, 256, 128]

SK = 32.0   # Wk, Wr scale
SQ = 32.0   # Wq scale
SV = 32.0   # Wv scale
SO = 64.0   # Wo scale
SBIA = 256.0  # wbias scale
ONES = 2.0  # vq ones value -> Z rows hold 2*Z
BROW = 4.0  # qsb bias-row constant: 256*4 = 1024 = (SQ*SK)
EXP_SCALE = 2.0 ** -13   # 1/(SQ*SK*8)
OUT_SCALE = 2.0 ** -10   # 1/(16*SO);  ctxsb holds 16*ctx (32/ONES)

_CACHE = {}

FP8 = ml_dtypes.float8_e4m3


def _build():
    dt = mybir.dt
    f32, fp8 = dt.float32, dt.float8e4
    nc = bacc.Bacc("TRN2", target_bir_lowering=False, debug=False, num_devices=8)

    qt_d = nc.dram_tensor("qt", [128, 8, 512], fp8, kind="ExternalInput")
    kvrl_d = nc.dram_tensor("kvrl", [128, 16, TK], fp8, kind="ExternalInput")
    wkr_d = nc.dram_tensor("wkr", [128, 8, 8, 2, 128], fp8, kind="ExternalInput")
    wq_d = nc.dram_tensor("wq", [128, 8, 4, 2, 128], fp8, kind="ExternalInput")
    wv_d = nc.dram_tensor("wv", [2, 128, 4, 2, 512], fp8, kind="ExternalInput")
    wo_d = nc.dram_tensor("wo", [128, 4, 2, 1024], fp8, kind="ExternalInput")
    wb_d = nc.dram_tensor("wb", [128, 8, 2, 16], fp8, kind="ExternalInput")
    msk_d = nc.dram_tensor("msk", [128, 8, 128], fp8, kind="ExternalInput")
    cst_d = nc.dram_tensor("cst", [16, 2, 1536], fp8, kind="ExternalInput")
    qres_d = nc.dram_tensor("qres", [4, 128, 1024], f32, kind="ExternalInput")
    gam_d = nc.dram_tensor("gam", [1024], f32, kind="ExternalInput")
    bet_d = nc.dram_tensor("bet", [1024], f32, kind="ExternalInput")
    out_d = nc.dram_tensor("out", [4, 128, 1024], f32, kind="ExternalOutput")

    Alu = mybir.AluOpType
    Act = mybir.ActivationFunctionType
    DR = mybir.MatmulPerfMode.DoubleRow

    with tile.TileContext(nc) as tc:
        import contextlib
        ctx = contextlib.ExitStack()
        with ctx:
            inp = ctx.enter_context(tc.tile_pool(name="inp", bufs=1))
            wts = ctx.enter_context(tc.tile_pool(name="wts", bufs=2))
            rawp = ctx.enter_context(tc.tile_pool(name="rawp", bufs=3))
            esp = ctx.enter_context(tc.tile_pool(name="esp", bufs=4))
            zp = ctx.enter_context(tc.tile_pool(name="zp", bufs=2))
            qrp = ctx.enter_context(tc.tile_pool(name="qrp", bufs=2))
            xp = ctx.enter_context(tc.tile_pool(name="xp", bufs=3))
            prod = ctx.enter_context(tc.tile_pool(name="prod", bufs=2, space="PSUM"))
            scps = ctx.enter_context(tc.tile_pool(name="scps", bufs=2, space="PSUM"))
            ctxps = ctx.enter_context(tc.tile_pool(name="ctxps", bufs=2, space="PSUM"))

            # ---------------- resident tiles + loads ----------------
            kvrl = inp.tile([128, 16, TK], fp8)
            qt = inp.tile([128, 8, 512], fp8)
            # kr stationary, DR layout + bias row:
            #   parts 0-31: h-even dim pairs, part 32: h-even bias row
            #   parts 64-95: h-odd dim pairs, part 96: h-odd bias row
            krsb = inp.tile([128, 8, 2, TK], fp8)
            qsb = inp.tile([128, 8, 2, 512], fp8)
            vq = inp.tile([128, 16, 6, 2, 128], fp8)
            ctxsb = inp.tile([128, 4, 2, 512], fp8)
            wo = inp.tile([128, 4, 2, 1024], fp8)
            wbias = inp.tile([128, 8, 2, 16], fp8)
            wkr_all = inp.tile([128, 8, 8, 2, 128], fp8)
            wq_all = inp.tile([128, 8, 4, 2, 128], fp8)
            msk = inp.tile([128, 8, 128], fp8)
            gam = inp.tile([128, 1024], f32)
            bet = inp.tile([128, 1024], f32)
            eps_t = inp.tile([128, 1], f32)
            dummy = inp.tile([128, 1], f32)

            # load order = first-consumer order: q path, then kr pair 0,
            # then the rest (DMA transfers serialize on the engine pool)
            nc.sync.dma_start(qt[:], qt_d[:])
            nc.sync.dma_start(wq_all[:, 0:1], wq_d[:, 0:1])
            nc.sync.dma_start(kvrl[:, :, 0:512], kvrl_d[:, :, 0:512])
            nc.sync.dma_start(wkr_all[:, 0:2], wkr_d[:, 0:2])
            nc.sync.dma_start(wbias[:], wb_d[:])
            nc.sync.dma_start(wq_all[:, 1:8], wq_d[:, 1:8])
            wv0 = None  # placeholder, real tile allocated below
            for c3 in (1, 2):
                cs = slice(512 * c3, 512 * c3 + 512)
                nc.sync.dma_start(kvrl[:, :, cs], kvrl_d[:, :, cs])
            nc.vector.memset(eps_t[:], 1e-5)
            # warm the Exp activation table off the critical path
            nc.scalar.activation(dummy[:], eps_t[:], Act.Exp)
            # vq ones halves (Z accumulator rows), one-time
            nc.gpsimd.memset(vq[:, :, :, :, 64:128], ONES)
            # qsb bias rows: part 32/96 slot0 = BROW, slot1 = 0; krsb bias
            # slot1 rows zeroed (slot0 written later by bias pass DMAs)
            nc.sync.dma_start(qsb[32:33, :, :, :], cst_d[0:8, :, 0:512])
            nc.sync.dma_start(qsb[96:97, :, :, :], cst_d[8:16, :, 0:512])
            nc.sync.dma_start(krsb[32:33, :, 1, :], cst_d[0:8, 1, :])
            nc.sync.dma_start(krsb[96:97, :, 1, :], cst_d[8:16, 1, :])

            # ---------------- production passes ----------------
            def emit_bias(c3):
                # 256*(u.k+v.r) -> krsb row 32/96
                cs = slice(512 * c3, 512 * c3 + 512)
                bp = prod.tile([128, 512], f32, tag="prod")
                for cp in range(8):
                    nc.tensor.matmul(bp[0:16, :], wbias[:, cp, :, :],
                                     kvrl[:, 2 * cp:2 * cp + 2, cs],
                                     start=(cp == 0), stop=(cp == 7),
                                     perf_mode=DR)
                braw = rawp.tile([16, 512], fp8, tag="braw")
                nc.vector.tensor_copy(braw[:], bp[0:16, :])
                nc.sync.dma_start(krsb[32:33, :, 0, cs], braw[0:8, :])
                nc.sync.dma_start(krsb[96:97, :, 0, cs], braw[8:16, :])

            def emit_q(pr):
                qp = prod.tile([128, 512], f32, tag="prod")
                for cp in range(4):
                    nc.tensor.matmul(qp[:], wq_all[:, pr, cp, :, :],
                                     qt[:, 2 * cp:2 * cp + 2, :],
                                     start=(cp == 0), stop=(cp == 3),
                                     perf_mode=DR)
                qraw = rawp.tile([128, 512], fp8, tag="qraw")
                nc.vector.tensor_copy(qraw[:], qp[:])
                for s in (0, 1):
                    nc.sync.dma_start(qsb[64 * s:64 * s + 32, pr, :, :],
                                      qraw[64 * s:64 * s + 64, :])

            def emit_v(o, trange):
                for t in trange:
                    vp = prod.tile([128, 512], f32, tag="prod")
                    for cp in range(4):
                        nc.tensor.matmul(vp[:],
                                         kvrl[:, 2 * cp:2 * cp + 2,
                                              128 * t:128 * t + 128],
                                         wvt[o][:, cp, :, :],
                                         start=(cp == 0), stop=(cp == 3),
                                         perf_mode=DR)
                    nc.vector.tensor_copy(
                        vq[:, 8 * o:8 * o + 8, t // 2, t % 2, 0:64],
                        vp[:].rearrange("p (h f) -> p h f", h=8))

            def emit_kr_chunk(pr, krraw, c3, chunked):
                cs = slice(512 * c3, 512 * c3 + 512)
                kp = prod.tile([128, 512], f32, tag="prod")
                for cp in range(8):
                    nc.tensor.matmul(kp[:], wkr_all[:, pr, cp, :, :],
                                     kvrl[:, 2 * cp:2 * cp + 2, cs],
                                     start=(cp == 0), stop=(cp == 7),
                                     perf_mode=DR)
                nc.vector.tensor_copy(krraw[:, cs], kp[:])
                if chunked:
                    for s in (0, 1):
                        nc.sync.dma_start(
                            krsb[64 * s:64 * s + 32, pr, :, cs],
                            krraw[64 * s:64 * s + 64, cs])

            def emit_kr(pr, chunked=False):
                krraw = rawp.tile([128, TK], fp8, tag="krraw", name="krraw")
                for c3 in range(3):
                    emit_kr_chunk(pr, krraw, c3, chunked)
                if not chunked:
                    for s in (0, 1):
                        nc.sync.dma_start(krsb[64 * s:64 * s + 32, pr, :, :],
                                          krraw[64 * s:64 * s + 64, :])

            wvt = {}
            wvt[0] = wts.tile([128, 4, 2, 512], fp8, tag="wv", name="wv0")
            nc.sync.dma_start(wvt[0][:], wv_d[0])
            nc.sync.dma_start(msk[:], msk_d[:])
            emit_q(0)
            krraw0 = rawp.tile([128, TK], fp8, tag="krraw", name="krraw0")

            def load_wv1():
                wvt[1] = wts.tile([128, 4, 2, 512], fp8, tag="wv", name="wv1")
                nc.sync.dma_start(wvt[1][:], wv_d[1])

            # ---------------- head loop (software-pipelined) ----------
            def emit_score_group(pr, s, es, gi):
                pb = 64 * s   # partition base for this head's kr/q rows
                w = PAIR_W[gi]
                off = 512 - w
                sp = scps.tile([128, 1024], f32, tag="sc", name="sp")
                for tt in range(2):
                    t = 2 * gi + tt
                    nc.tensor.matmul(
                        sp[:, 512 * tt + off:512 * tt + 512],
                        krsb[pb:pb + 33, pr, :, 128 * t:128 * t + 128],
                        qsb[pb:pb + 33, pr, :, off:512],
                        start=True, stop=True, perf_mode=DR)
                # exp over both tiles of the pair: [128, 2, w]
                spap = sp[:]
                src = bass.AP(tensor=spap.tensor, offset=spap.offset + off,
                              ap=[list(spap.ap[0]), [512, 2], [1, w]])
                nc.scalar.activation(es[:, gi, :, off:512], src, Act.Exp,
                                     scale=EXP_SCALE)

            def emit_mask(es, mask_eng=None):
                # data-driven diagonal masks on pairs 2..5 (one strided op)
                esap = es[:]
                es_m = bass.AP(tensor=esap.tensor, offset=esap.offset + 2048,
                               ap=[list(esap.ap[0]), [1152, 4], [512, 2], [1, 128]])
                mkap = msk[:]
                mk_m = bass.AP(tensor=mkap.tensor, offset=mkap.offset,
                               ap=[list(mkap.ap[0]), [256, 4], [128, 2], [1, 128]])
                (mask_eng or nc.vector).tensor_tensor(es_m, es_m, mk_m, Alu.mult)

            def emit_scores(pr, s, mask_eng=None):
                es = esp.tile([128, 6, 2, 512], fp8, tag="es", name="es")
                for gi in range(6):
                    emit_score_group(pr, s, es, gi)
                emit_mask(es, mask_eng)
                return es

            def emit_ctx(pr, s, es):
                h = 2 * pr + s
                cps = ctxps.tile([128, 512], f32, tag="ctx")
                for gi in range(6):
                    w = PAIR_W[gi]
                    off = 512 - w
                    nc.tensor.matmul(cps[:, off:512], vq[:, h, gi, :, :],
                                     es[:, gi, :, off:512],
                                     start=(gi == 0), stop=(gi == 5),
                                     skip_group_check=True, perf_mode=DR)
                zr = zp.tile([128, 512], f32, tag="z")
                nc.vector.reciprocal(zr[0:64, :], cps[64:128, :])
                nc.vector.tensor_tensor(
                    ctxsb[64 * s:64 * s + 64, h // 4, (h // 2) % 2, :],
                    cps[0:64, :], zr[0:64, :], Alu.mult)

            extras = {
                0: [lambda: emit_kr(1)],
                1: [lambda: emit_v(0, range(6, 12))],
                2: [lambda: (emit_q(2), emit_kr(2))],
                4: [lambda: (emit_q(3), emit_kr(3)), load_wv1],
                6: [lambda: emit_v(1, range(0, 6)),
                    lambda: (emit_q(4), emit_kr(4))],
                8: [lambda: emit_v(1, range(6, 12)),
                    lambda: (emit_q(5), emit_kr(5))],
                10: [lambda: (emit_q(6), emit_kr(6))],
                12: [lambda: (emit_q(7), emit_kr(7))],
            }
            # head (0,0): score groups interleaved with the kr0/bias tk
            # chunks they depend on, so exp starts as soon as chunk 0 lands
            es0 = esp.tile([128, 6, 2, 512], fp8, tag="es", name="es0")
            for c3 in range(3):
                emit_kr_chunk(0, krraw0, c3, chunked=True)
                emit_bias(c3)
                emit_score_group(0, 0, es0, 2 * c3)
                emit_score_group(0, 0, es0, 2 * c3 + 1)
            emit_mask(es0)
            # deferred bulk loads (transfers queue behind the critical ones)
            nc.sync.dma_start(wkr_all[:, 2:8], wkr_d[:, 2:8])
            emit_q(1)
            emit_v(0, range(6))

            heads = [(pr, s) for pr in range(8) for s in (0, 1)]
            pending = (0, 0, es0)
            for idx, (pr, s) in enumerate(heads):
                es = emit_scores(pr, s) if idx > 0 else None
                for fn_ in extras.get(idx, []):
                    fn_()
                if idx > 0:
                    emit_ctx(*pending)
                    pending = (pr, s, es)
            emit_ctx(*pending)

            # ---------------- output projection + residual + layernorm
            nc.sync.dma_start(wo[:], wo_d[:])
            _g, _b = gam_d.ap(), bet_d.ap()
            gam_b = bass.AP(tensor=_g.tensor, offset=_g.offset,
                            ap=[[0, 128], [1, 1024]])
            bet_b = bass.AP(tensor=_b.tensor, offset=_b.offset,
                            ap=[[0, 128], [1, 1024]])
            nc.sync.dma_start(gam[:], gam_b)
            nc.sync.dma_start(bet[:], bet_b)
            for tqt in range(4):
                qr = qrp.tile([128, 1024], f32, tag="qr")
                nc.sync.dma_start(qr[:], qres_d[tqt])
                xsb = xp.tile([128, 1024], f32, tag="x")
                tq_sl = slice(128 * tqt, 128 * tqt + 128)
                for dh in range(2):
                    d_sl = slice(512 * dh, 512 * dh + 512)
                    wp = prod.tile([128, 512], f32, tag="prod")
                    for c in range(4):
                        nc.tensor.matmul(wp[:], ctxsb[:, c, :, tq_sl],
                                         wo[:, c, :, d_sl],
                                         start=(c == 0), stop=(c == 3),
                                         perf_mode=DR)
                    nc.vector.scalar_tensor_tensor(
                        xsb[:, d_sl], wp[:], OUT_SCALE, qr[:, d_sl],
                        Alu.mult, Alu.add)
                stats = xp.tile([128, 2, 6], f32, tag="st")
                for g in range(2):
                    nc.vector.bn_stats(stats[:, g, :],
                                       xsb[:, 512 * g:512 * g + 512])
                mv = xp.tile([128, 2], f32, tag="mv")
                nc.vector.bn_aggr(mv[:], stats[:])
                nc.scalar.activation(mv[:, 1:2], mv[:, 1:2], Act.Sqrt,
                                     bias=eps_t[:], scale=1.0)
                nc.vector.reciprocal(mv[:, 1:2], mv[:, 1:2])
                o = xp.tile([128, 1024], f32, tag="o")
                for eng, hsl in ((nc.vector, slice(0, 512)),
                                 (nc.gpsimd, slice(512, 1024))):
                    eng.tensor_scalar(o[:, hsl], xsb[:, hsl], mv[:, 0:1],
                                      mv[:, 1:2],
                                      op0=Alu.subtract, op1=Alu.mult)
                    eng.tensor_tensor(o[:, hsl], o[:, hsl], gam[:, hsl],
                                      Alu.mult)
                    eng.tensor_tensor(o[:, hsl], o[:, hsl], bet[:, hsl],
                                      Alu.add)
                nc.sync.dma_start(out_d[tqt], o[:])

    nc.compile()
    return nc


def _tri128():
    r = np.arange(128)
    return (r[:, None] <= r[None, :]).astype(np.float32)


def _fp8(x):
    return np.clip(np.asarray(x, dtype=np.float32), -240.0, 240.0).astype(FP8)


def _dr_pack_w(Wblk, cols_scale):
    """[1024, C] weight block -> [128, 4, 2, C] DR chunk-pair layout."""
    C = Wblk.shape[1]
    return np.ascontiguousarray(
        (Wblk * cols_scale).reshape(4, 2, 128, C).transpose(2, 0, 1, 3))


def _prep_core(c, query, key_value, relative, Wq, Wk, Wv, Wr, Wo, u, v,
               gamma, beta):
    b, half = c // 2, c % 2
    slots = QSLOTS[half]
    rows = np.concatenate([np.arange(128 * qi, 128 * qi + 128) for qi in slots])
    qloc = np.ascontiguousarray(query[b][rows])            # [512, 1024]
    qt = _fp8(np.ascontiguousarray(
        qloc.T.reshape(8, 128, 512).transpose(1, 0, 2)))
    kvt = key_value[b].T.reshape(8, 128, TK).transpose(1, 0, 2)
    rlt = relative[b].T.reshape(8, 128, TK).transpose(1, 0, 2)
    kvrl = _fp8(np.concatenate([kvt, rlt], axis=1))        # [128, 16, TK]

    wkr = np.empty((128, 8, 8, 2, 128), dtype=FP8)
    wq = np.empty((128, 8, 4, 2, 128), dtype=FP8)
    for pr in range(8):
        cs = slice(128 * pr, 128 * pr + 128)
        wkr[:, pr, 0:4] = _fp8(_dr_pack_w(Wk[:, cs], SK))
        wkr[:, pr, 4:8] = _fp8(_dr_pack_w(Wr[:, cs], SK))
        wq[:, pr] = _fp8(_dr_pack_w(Wq[:, cs], SQ))
    wv = np.empty((2, 128, 4, 2, 512), dtype=FP8)
    for o in range(2):
        wv[o] = _fp8(_dr_pack_w(Wv[:, 512 * o:512 * o + 512], SV))
    # wo: [p, c, j, :] = Wo[64*(4c+2j+p//64) + p%64, :] * SO
    wo = _fp8(np.ascontiguousarray(
        (Wo * SO).reshape(4, 2, 2, 64, 1024).transpose(2, 3, 0, 1, 4)
        .reshape(128, 4, 2, 1024)))
    # wbias: per-head key-bias weights  [Wk_h @ u ; Wr_h @ v]
    WU = np.einsum('dhk,k->dh', Wk.reshape(1024, 16, 64), u)   # [1024, 16]
    WV = np.einsum('dhk,k->dh', Wr.reshape(1024, 16, 64), v)
    perm = [2 * i for i in range(8)] + [2 * i + 1 for i in range(8)]
    WU, WV = WU[:, perm], WV[:, perm]
    wb = np.empty((128, 8, 2, 16), dtype=FP8)
    wb[:, 0:4] = _fp8(WU.reshape(4, 2, 128, 16).transpose(2, 0, 1, 3) * SBIA)
    wb[:, 4:8] = _fp8(WV.reshape(4, 2, 128, 16).transpose(2, 0, 1, 3) * SBIA)

    cst = np.zeros((16, 2, 1536), dtype=FP8)
    cst[:, 0, :] = FP8(BROW)

    tri = _tri128()
    masks = np.empty((8, 128, 128), dtype=np.float32)
    for p, (t, s) in enumerate(MASK_POS):
        qi = slots[s]
        if qi + 4 > t:
            masks[p] = 1.0
        elif qi + 4 == t:
            masks[p] = tri
        else:
            masks[p] = 0.0
    qres = np.ascontiguousarray(qloc.reshape(4, 128, 1024)).astype(np.float32)
    return {
        "qt": qt, "kvrl": kvrl, "wkr": wkr, "wq": wq, "wv": wv, "wo": wo,
        "wb": wb, "msk": _fp8(np.ascontiguousarray(masks.transpose(1, 0, 2))),
        "cst": cst, "qres": qres,
        "gam": gamma.astype(np.float32), "bet": beta.astype(np.float32),
    }


def kernel(query, key_value, relative, mask, Wq, Wk, Wv, Wr, Wo, u, v,
           gamma, beta):
    query = np.asarray(query, dtype=np.float32)
    key_value = np.asarray(key_value, dtype=np.float32)
    relative = np.asarray(relative, dtype=np.float32)
    Wq = np.asarray(Wq, dtype=np.float32)
    Wk = np.asarray(Wk, dtype=np.float32)
    Wv = np.asarray(Wv, dtype=np.float32)
    Wr = np.asarray(Wr, dtype=np.float32)
    Wo = np.asarray(Wo, dtype=np.float32)
    u = np.asarray(u, dtype=np.float32)
    v = np.asarray(v, dtype=np.float32)
    gamma = np.asarray(gamma, dtype=np.float32)
    beta = np.asarray(beta, dtype=np.float32)

    if "nc" not in _CACHE:
        _CACHE["nc"] = _build()
    nc = _CACHE["nc"]

    in_maps = [
        _prep_core(c, query, key_value, relative, Wq, Wk, Wv, Wr, Wo, u, v,
                   gamma, beta)
        for c in range(8)
    ]
    import os
    trace = bool(int(os.environ.get("KERNEL_TRACE", "0")))
    kwargs = {}
    if trace:
        kwargs = {"trace": True, "trace_cores": [0]}
    res = run_bass_kernel_spmd(nc, in_maps, core_ids=list(range(8)), **kwargs)
    _CACHE["last_result"] = res

    out = np.empty((B, TQ, D), dtype=np.float32)
    for c in range(8):
        b, half = c // 2, c % 2
        o = res.results[c]["out"].reshape(512, 1024)
        rows = np.concatenate(
            [np.arange(128 * qi, 128 * qi + 128) for qi in QSLOTS[half]])
        out[b][rows] = o
    return out


# revision 61
# speedup vs baseline: 1.0184x; 1.0184x over previous
"""Transformer-XL attention kernel for 8 TRN2 NeuronCores — fp8 DoubleRow.

Sharding: data-parallel over batch B=4 x 2-way split of query rows
(interleaved 128-row tiles for mask balance). No collectives.

Math restructure vs the bf16 baseline:
  (q+u)k^T + (q+v)r^T  =  q(k+r)^T + [u.k + v.r]     (per head)
The second term is a per-key bias row, produced by a skinny matmul and
folded into the score matmul as an extra contraction row (so the exp
activation needs no per-tile bias operand and can batch 2 tiles).

All matmuls run fp8e4 with perf_mode=DoubleRow (2 contraction rows per
partition, operands shaped [P, 2, N]):
  - projections (k+r / q / v / bias): weights host-packed in DR layout
  - scores: contraction 64+1 (kr dims + bias row); kr/q psum evictions
    are reshuffled into the DR layout by SBUF->SBUF DMAs (adjacent-dim
    pairing (2d, 2d+1) so each reshuffle is one contiguous DMA)
  - ctx: DR pairs adjacent tk tiles (union widths are pair-equal)
  - out-proj: ctx eviction writes the DR layout directly

Scales (host-staged): Wk,Wr,Wq,Wv x32, Wo x64, wbias x256; exp scale
2^-13 recovers true logits/sqrt(dv); vq ones=2.0; out evict 2^-10.

Engine budget (TimelineSim): ACT ~82us (exp, the critical stream, kept
saturated by pipelining scores(h+1) ahead of ctx(h)), DVE ~94us (psum
evictions + masks + normalize + LN; GPSIMD cannot read PSUM), PE ~65us,
Pool ~23us (SBUF-only ops). Head 0 is interleaved with the kr/bias tk
chunks it consumes so the first exp fires as soon as chunk 0 lands.
"""

import numpy as np
import ml_dtypes

import concourse.bass as bass
from concourse import bacc
import concourse.mybir as mybir
import concourse.tile as tile
from concourse.bass_utils import run_bass_kernel_spmd

B, TQ, TK, D, H, DV = 4, 1024, 1536, 1024, 16, 64
NTK = 12
QSLOTS = {0: [0, 3, 4, 7], 1: [1, 2, 5, 6]}
# union structural widths per tk tile (512 - 128*fp)
FP_UNION = [0, 0, 0, 0, 0, 0, 1, 1, 2, 2, 3, 3]
MASK_POS = [(4, 0), (5, 0), (6, 1), (7, 1), (8, 2), (9, 2), (10, 3), (11, 3)]
# pair widths for tk-tile pairs (0,1),(2,3),...,(10,11) — equal within pair
PAIR_W = [512, 512, 512, 384, 256, 128]

SK = 32.0   # Wk, Wr scale
SQ = 32.0   # Wq scale
SV = 32.0   # Wv scale
SO = 64.0   # Wo scale
SBIA = 256.0  # wbias scale
ONES = 2.0  # vq ones value -> Z rows hold 2*Z
BROW = 4.0  # qsb bias-row constant: 256*4 = 1024 = (SQ*SK)
EXP_SCALE = 2.0 ** -13   # 1/(SQ*SK*8)
OUT_SCALE = 2.0 ** -10   # 1/(16*SO);  ctxsb holds 16*ctx (32/ONES)

_CACHE = {}

FP8 = ml_dtypes.float8_e4m3


def _build():
    dt = mybir.dt
    f32, fp8 = dt.float32, dt.float8e4
    nc = bacc.Bacc("TRN2", target_bir_lowering=False, debug=False, num_devices=8)

    qt_d = nc.dram_tensor("qt", [128, 8, 512], fp8, kind="ExternalInput")
    kvrl_d = nc.dram_tensor("kvrl", [128, 16, TK], fp8, kind="ExternalInput")
    wkr_d = nc.dram_tensor("wkr", [128, 8, 8, 2, 128], fp8, kind="ExternalInput")
    wq_d = nc.dram_tensor("wq", [128, 8, 4, 2, 128], fp8, kind="ExternalInput")
    wv_d = nc.dram_tensor("wv", [2, 128, 4, 2, 512], fp8, kind="ExternalInput")
    wo_d = nc.dram_tensor("wo", [128, 4, 2, 1024], fp8, kind="ExternalInput")
    wb_d = nc.dram_tensor("wb", [128, 8, 2, 16], fp8, kind="ExternalInput")
    msk_d = nc.dram_tensor("msk", [128, 8, 128], fp8, kind="ExternalInput")
    cst_d = nc.dram_tensor("cst", [16, 2, 1536], fp8, kind="ExternalInput")
    qres_d = nc.dram_tensor("qres", [4, 128, 1024], f32, kind="ExternalInput")
    gam_d = nc.dram_tensor("gam", [1024], f32, kind="ExternalInput")
    bet_d = nc.dram_tensor("bet", [1024], f32, kind="ExternalInput")
    out_d = nc.dram_tensor("out", [4, 128, 1024], f32, kind="ExternalOutput")

    Alu = mybir.AluOpType
    Act = mybir.ActivationFunctionType
    DR = mybir.MatmulPerfMode.DoubleRow

    with tile.TileContext(nc) as tc:
        import contextlib
        ctx = contextlib.ExitStack()
        with ctx:
            inp = ctx.enter_context(tc.tile_pool(name="inp", bufs=1))
            wts = ctx.enter_context(tc.tile_pool(name="wts", bufs=2))
            rawp = ctx.enter_context(tc.tile_pool(name="rawp", bufs=3))
            esp = ctx.enter_context(tc.tile_pool(name="esp", bufs=4))
            zp = ctx.enter_context(tc.tile_pool(name="zp", bufs=2))
            qrp = ctx.enter_context(tc.tile_pool(name="qrp", bufs=2))
            xp = ctx.enter_context(tc.tile_pool(name="xp", bufs=3))
            prod = ctx.enter_context(tc.tile_pool(name="prod", bufs=2, space="PSUM"))
            scps = ctx.enter_context(tc.tile_pool(name="scps", bufs=2, space="PSUM"))
            ctxps = ctx.enter_context(tc.tile_pool(name="ctxps", bufs=2, space="PSUM"))

            # ---------------- resident tiles + loads ----------------
            kvrl = inp.tile([128, 16, TK], fp8)
            qt = inp.tile([128, 8, 512], fp8)
            # kr stationary, DR layout + bias row:
            #   parts 0-31: h-even dim pairs, part 32: h-even bias row
            #   parts 64-95: h-odd dim pairs, part 96: h-odd bias row
            krsb = inp.tile([128, 8, 2, TK], fp8)
            qsb = inp.tile([128, 8, 2, 512], fp8)
            vq = inp.tile([128, 16, 6, 2, 128], fp8)
            ctxsb = inp.tile([128, 4, 2, 512], fp8)
            wo = inp.tile([128, 4, 2, 1024], fp8)
            wbias = inp.tile([128, 8, 2, 16], fp8)
            wkr_all = inp.tile([128, 8, 8, 2, 128], fp8)
            wq_all = inp.tile([128, 8, 4, 2, 128], fp8)
            msk = inp.tile([128, 8, 128], fp8)
            gam = inp.tile([128, 1024], f32)
            bet = inp.tile([128, 1024], f32)
            eps_t = inp.tile([128, 1], f32)
            dummy = inp.tile([128, 1], f32)

            # load order = first-consumer order: q path, then kr pair 0,
            # then the rest (DMA transfers serialize on the engine pool)
            nc.sync.dma_start(qt[:], qt_d[:])
            nc.sync.dma_start(wq_all[:, 0:1], wq_d[:, 0:1])
            nc.sync.dma_start(kvrl[:, :, 0:512], kvrl_d[:, :, 0:512])
            nc.sync.dma_start(wkr_all[:, 0:2], wkr_d[:, 0:2])
            nc.sync.dma_start(wbias[:], wb_d[:])
            nc.sync.dma_start(wq_all[:, 1:8], wq_d[:, 1:8])
            wv0 = None  # placeholder, real tile allocated below
            for c3 in (1, 2):
                cs = slice(512 * c3, 512 * c3 + 512)
                nc.sync.dma_start(kvrl[:, :, cs], kvrl_d[:, :, cs])
            nc.vector.memset(eps_t[:], 1e-5)
            # warm the Exp activation table off the critical path
            nc.scalar.activation(dummy[:], eps_t[:], Act.Exp)
            # vq ones halves (Z accumulator rows), one-time
            nc.gpsimd.memset(vq[:, :, :, :, 64:128], ONES)
            # qsb bias rows: part 32/96 slot0 = BROW, slot1 = 0; krsb bias
            # slot1 rows zeroed (slot0 written later by bias pass DMAs)
            nc.sync.dma_start(qsb[32:33, :, :, :], cst_d[0:8, :, 0:512])
            nc.sync.dma_start(qsb[96:97, :, :, :], cst_d[8:16, :, 0:512])
            nc.sync.dma_start(krsb[32:33, :, 1, :], cst_d[0:8, 1, :])
            nc.sync.dma_start(krsb[96:97, :, 1, :], cst_d[8:16, 1, :])

            # ---------------- production passes ----------------
            def emit_bias(c3):
                # 256*(u.k+v.r) -> krsb row 32/96
                cs = slice(512 * c3, 512 * c3 + 512)
                bp = prod.tile([128, 512], f32, tag="prod")
                for cp in range(8):
                    nc.tensor.matmul(bp[0:16, :], wbias[:, cp, :, :],
                                     kvrl[:, 2 * cp:2 * cp + 2, cs],
                                     start=(cp == 0), stop=(cp == 7),
                                     perf_mode=DR)
                braw = rawp.tile([16, 512], fp8, tag="braw")
                nc.vector.tensor_copy(braw[:], bp[0:16, :])
                nc.sync.dma_start(krsb[32:33, :, 0, cs], braw[0:8, :])
                nc.sync.dma_start(krsb[96:97, :, 0, cs], braw[8:16, :])

            def emit_q(pr):
                qp = prod.tile([128, 512], f32, tag="prod")
                for cp in range(4):
                    nc.tensor.matmul(qp[:], wq_all[:, pr, cp, :, :],
                                     qt[:, 2 * cp:2 * cp + 2, :],
                                     start=(cp == 0), stop=(cp == 3),
                                     perf_mode=DR)
                qraw = rawp.tile([128, 512], fp8, tag="qraw")
                nc.vector.tensor_copy(qraw[:], qp[:])
                for s in (0, 1):
                    nc.sync.dma_start(qsb[64 * s:64 * s + 32, pr, :, :],
                                      qraw[64 * s:64 * s + 64, :])

            def emit_v(o, trange):
                for t in trange:
                    vp = prod.tile([128, 512], f32, tag="prod")
                    for cp in range(4):
                        nc.tensor.matmul(vp[:],
                                         kvrl[:, 2 * cp:2 * cp + 2,
                                              128 * t:128 * t + 128],
                                         wvt[o][:, cp, :, :],
                                         start=(cp == 0), stop=(cp == 3),
                                         perf_mode=DR)
                    nc.vector.tensor_copy(
                        vq[:, 8 * o:8 * o + 8, t // 2, t % 2, 0:64],
                        vp[:].rearrange("p (h f) -> p h f", h=8))

            def emit_kr_chunk(pr, krraw, c3, chunked):
                cs = slice(512 * c3, 512 * c3 + 512)
                kp = prod.tile([128, 512], f32, tag="prod")
                for cp in range(8):
                    nc.tensor.matmul(kp[:], wkr_all[:, pr, cp, :, :],
                                     kvrl[:, 2 * cp:2 * cp + 2, cs],
                                     start=(cp == 0), stop=(cp == 7),
                                     perf_mode=DR)
                nc.vector.tensor_copy(krraw[:, cs], kp[:])
                if chunked:
                    for s in (0, 1):
                        nc.sync.dma_start(
                            krsb[64 * s:64 * s + 32, pr, :, cs],
                            krraw[64 * s:64 * s + 64, cs])

            def emit_kr(pr, chunked=False):
                krraw = rawp.tile([128, TK], fp8, tag="krraw", name="krraw")
                for c3 in range(3):
                    emit_kr_chunk(pr, krraw, c3, chunked)
                if not chunked:
                    for s in (0, 1):
                        nc.sync.dma_start(krsb[64 * s:64 * s + 32, pr, :, :],
                                          krraw[64 * s:64 * s + 64, :])

            wvt = {}
            wvt[0] = wts.tile([128, 4, 2, 512], fp8, tag="wv", name="wv0")
            nc.sync.dma_start(wvt[0][:], wv_d[0])
            nc.sync.dma_start(msk[:], msk_d[:])
            emit_q(0)
            krraw0 = rawp.tile([128, TK], fp8, tag="krraw", name="krraw0")

            def load_wv1():
                wvt[1] = wts.tile([128, 4, 2, 512], fp8, tag="wv", name="wv1")
                nc.sync.dma_start(wvt[1][:], wv_d[1])

            # ---------------- head loop (software-pipelined) ----------
            def emit_score_group(pr, s, es, gi):
                pb = 64 * s   # partition base for this head's kr/q rows
                w = PAIR_W[gi]
                off = 512 - w
                sp = scps.tile([128, 1024], f32, tag="sc", name="sp")
                for tt in range(2):
                    t = 2 * gi + tt
                    nc.tensor.matmul(
                        sp[:, 512 * tt + off:512 * tt + 512],
                        krsb[pb:pb + 33, pr, :, 128 * t:128 * t + 128],
                        qsb[pb:pb + 33, pr, :, off:512],
                        start=True, stop=True, perf_mode=DR)
                # exp over both tiles of the pair: [128, 2, w]
                spap = sp[:]
                src = bass.AP(tensor=spap.tensor, offset=spap.offset + off,
                              ap=[list(spap.ap[0]), [512, 2], [1, w]])
                nc.scalar.activation(es[:, gi, :, off:512], src, Act.Exp,
                                     scale=EXP_SCALE)

            def emit_mask(es, mask_eng=None):
                # data-driven diagonal masks on pairs 2..5 (one strided op)
                esap = es[:]
                es_m = bass.AP(tensor=esap.tensor, offset=esap.offset + 2048,
                               ap=[list(esap.ap[0]), [1152, 4], [512, 2], [1, 128]])
                mkap = msk[:]
                mk_m = bass.AP(tensor=mkap.tensor, offset=mkap.offset,
                               ap=[list(mkap.ap[0]), [256, 4], [128, 2], [1, 128]])
                (mask_eng or nc.vector).tensor_tensor(es_m, es_m, mk_m, Alu.mult)

            def emit_scores(pr, s, mask_eng=None):
                es = esp.tile([128, 6, 2, 512], fp8, tag="es", name="es")
                for gi in range(6):
                    emit_score_group(pr, s, es, gi)
                emit_mask(es, mask_eng)
                return es

            def emit_ctx(pr, s, es):
                h = 2 * pr + s
                cps = ctxps.tile([128, 512], f32, tag="ctx")
                for gi in range(6):
                    w = PAIR_W[gi]
                    off = 512 - w
                    nc.tensor.matmul(cps[:, off:512], vq[:, h, gi, :, :],
                                     es[:, gi, :, off:512],
                                     start=(gi == 0), stop=(gi == 5),
                                     skip_group_check=True, perf_mode=DR)
                zr = zp.tile([128, 512], f32, tag="z")
                nc.vector.reciprocal(zr[0:64, :], cps[64:128, :])
                nc.vector.tensor_tensor(
                    ctxsb[64 * s:64 * s + 64, h // 4, (h // 2) % 2, :],
                    cps[0:64, :], zr[0:64, :], Alu.mult)

            extras = {
                0: [lambda: emit_kr(1)],
                1: [lambda: emit_v(0, range(6, 12))],
                2: [lambda: (emit_q(2), emit_kr(2))],
                4: [lambda: (emit_q(3), emit_kr(3)), load_wv1],
                6: [lambda: emit_v(1, range(0, 6)),
                    lambda: (emit_q(4), emit_kr(4))],
                8: [lambda: emit_v(1, range(6, 12)),
                    lambda: (emit_q(5), emit_kr(5))],
                10: [lambda: (emit_q(6), emit_kr(6))],
                12: [lambda: (emit_q(7), emit_kr(7))],
            }
            # head (0,0): score groups interleaved with the kr0/bias tk
            # chunks they depend on, so exp starts as soon as chunk 0 lands
            es0 = esp.tile([128, 6, 2, 512], fp8, tag="es", name="es0")
            for c3 in range(3):
                emit_kr_chunk(0, krraw0, c3, chunked=True)
                emit_bias(c3)
                emit_score_group(0, 0, es0, 2 * c3)
                emit_score_group(0, 0, es0, 2 * c3 + 1)
            emit_mask(es0)
            # deferred bulk loads (transfers queue behind the critical ones)
            nc.sync.dma_start(wkr_all[:, 2:8], wkr_d[:, 2:8])
            emit_q(1)
            emit_v(0, range(6))

            heads = [(pr, s) for pr in range(8) for s in (0, 1)]
            pending = (0, 0, es0)
            for idx, (pr, s) in enumerate(heads):
                es = emit_scores(pr, s) if idx > 0 else None
                if idx > 0:
                    emit_ctx(*pending)
                for fn_ in extras.get(idx, []):
                    fn_()
                if idx > 0:
                    pending = (pr, s, es)
            emit_ctx(*pending)

            # ---------------- output projection + residual + layernorm
            nc.sync.dma_start(wo[:], wo_d[:])
            _g, _b = gam_d.ap(), bet_d.ap()
            gam_b = bass.AP(tensor=_g.tensor, offset=_g.offset,
                            ap=[[0, 128], [1, 1024]])
            bet_b = bass.AP(tensor=_b.tensor, offset=_b.offset,
                            ap=[[0, 128], [1, 1024]])
            nc.sync.dma_start(gam[:], gam_b)
            nc.sync.dma_start(bet[:], bet_b)
            for tqt in range(4):
                qr = qrp.tile([128, 1024], f32, tag="qr")
                nc.sync.dma_start(qr[:], qres_d[tqt])
                xsb = xp.tile([128, 1024], f32, tag="x")
                tq_sl = slice(128 * tqt, 128 * tqt + 128)
                for dh in range(2):
                    d_sl = slice(512 * dh, 512 * dh + 512)
                    wp = prod.tile([128, 512], f32, tag="prod")
                    for c in range(4):
                        nc.tensor.matmul(wp[:], ctxsb[:, c, :, tq_sl],
                                         wo[:, c, :, d_sl],
                                         start=(c == 0), stop=(c == 3),
                                         perf_mode=DR)
                    nc.vector.scalar_tensor_tensor(
                        xsb[:, d_sl], wp[:], OUT_SCALE, qr[:, d_sl],
                        Alu.mult, Alu.add)
                stats = xp.tile([128, 2, 6], f32, tag="st")
                for g in range(2):
                    nc.vector.bn_stats(stats[:, g, :],
                                       xsb[:, 512 * g:512 * g + 512])
                mv = xp.tile([128, 2], f32, tag="mv")
                nc.vector.bn_aggr(mv[:], stats[:])
                nc.scalar.activation(mv[:, 1:2], mv[:, 1:2], Act.Sqrt,
                                     bias=eps_t[:], scale=1.0)
                nc.vector.reciprocal(mv[:, 1:2], mv[:, 1:2])
                o = xp.tile([128, 1024], f32, tag="o")
                for eng, hsl in ((nc.vector, slice(0, 512)),
                                 (nc.gpsimd, slice(512, 1024))):
                    eng.tensor_scalar(o[:, hsl], xsb[:, hsl], mv[:, 0:1],
                                      mv[:, 1:2],
                                      op0=Alu.subtract, op1=Alu.mult)
                    eng.tensor_tensor(o[:, hsl], o[:, hsl], gam[:, hsl],
                                      Alu.mult)
                    eng.tensor_tensor(o[:, hsl], o[:, hsl], bet[:, hsl],
                                      Alu.add)
                nc.sync.dma_start(out_d[tqt], o[:])

    nc.compile()
    return nc


def _tri128():
    r = np.arange(128)
    return (r[:, None] <= r[None, :]).astype(np.float32)


def _fp8(x):
    return np.clip(np.asarray(x, dtype=np.float32), -240.0, 240.0).astype(FP8)


def _dr_pack_w(Wblk, cols_scale):
    """[1024, C] weight block -> [128, 4, 2, C] DR chunk-pair layout."""
    C = Wblk.shape[1]
    return np.ascontiguousarray(
        (Wblk * cols_scale).reshape(4, 2, 128, C).transpose(2, 0, 1, 3))


def _prep_core(c, query, key_value, relative, Wq, Wk, Wv, Wr, Wo, u, v,
               gamma, beta):
    b, half = c // 2, c % 2
    slots = QSLOTS[half]
    rows = np.concatenate([np.arange(128 * qi, 128 * qi + 128) for qi in slots])
    qloc = np.ascontiguousarray(query[b][rows])            # [512, 1024]
    qt = _fp8(np.ascontiguousarray(
        qloc.T.reshape(8, 128, 512).transpose(1, 0, 2)))
    kvt = key_value[b].T.reshape(8, 128, TK).transpose(1, 0, 2)
    rlt = relative[b].T.reshape(8, 128, TK).transpose(1, 0, 2)
    kvrl = _fp8(np.concatenate([kvt, rlt], axis=1))        # [128, 16, TK]

    wkr = np.empty((128, 8, 8, 2, 128), dtype=FP8)
    wq = np.empty((128, 8, 4, 2, 128), dtype=FP8)
    for pr in range(8):
        cs = slice(128 * pr, 128 * pr + 128)
        wkr[:, pr, 0:4] = _fp8(_dr_pack_w(Wk[:, cs], SK))
        wkr[:, pr, 4:8] = _fp8(_dr_pack_w(Wr[:, cs], SK))
        wq[:, pr] = _fp8(_dr_pack_w(Wq[:, cs], SQ))
    wv = np.empty((2, 128, 4, 2, 512), dtype=FP8)
    for o in range(2):
        wv[o] = _fp8(_dr_pack_w(Wv[:, 512 * o:512 * o + 512], SV))
    # wo: [p, c, j, :] = Wo[64*(4c+2j+p//64) + p%64, :] * SO
    wo = _fp8(np.ascontiguousarray(
        (Wo * SO).reshape(4, 2, 2, 64, 1024).transpose(2, 3, 0, 1, 4)
        .reshape(128, 4, 2, 1024)))
    # wbias: per-head key-bias weights  [Wk_h @ u ; Wr_h @ v]
    WU = np.einsum('dhk,k->dh', Wk.reshape(1024, 16, 64), u)   # [1024, 16]
    WV = np.einsum('dhk,k->dh', Wr.reshape(1024, 16, 64), v)
    perm = [2 * i for i in range(8)] + [2 * i + 1 for i in range(8)]
    WU, WV = WU[:, perm], WV[:, perm]
    wb = np.empty((128, 8, 2, 16), dtype=FP8)
    wb[:, 0:4] = _fp8(WU.reshape(4, 2, 128, 16).transpose(2, 0, 1, 3) * SBIA)
    wb[:, 4:8] = _fp8(WV.reshape(4, 2, 128, 16).transpose(2, 0, 1, 3) * SBIA)

    cst = np.zeros((16, 2, 1536), dtype=FP8)
    cst[:, 0, :] = FP8(BROW)

    tri = _tri128()
    masks = np.empty((8, 128, 128), dtype=np.float32)
    for p, (t, s) in enumerate(MASK_POS):
        qi = slots[s]
        if qi + 4 > t:
            masks[p] = 1.0
        elif qi + 4 == t:
            masks[p] = tri
        else:
            masks[p] = 0.0
    qres = np.ascontiguousarray(qloc.reshape(4, 128, 1024)).astype(np.float32)
    return {
        "qt": qt, "kvrl": kvrl, "wkr": wkr, "wq": wq, "wv": wv, "wo": wo,
        "wb": wb, "msk": _fp8(np.ascontiguousarray(masks.transpose(1, 0, 2))),
        "cst": cst, "qres": qres,
        "gam": gamma.astype(np.float32), "bet": beta.astype(np.float32),
    }


def kernel(query, key_value, relative, mask, Wq, Wk, Wv, Wr, Wo, u, v,
           gamma, beta):
    query = np.asarray(query, dtype=np.float32)
    key_value = np.asarray(key_value, dtype=np.float32)
    relative = np.asarray(relative, dtype=np.float32)
    Wq = np.asarray(Wq, dtype=np.float32)
    Wk = np.asarray(Wk, dtype=np.float32)
    Wv = np.asarray(Wv, dtype=np.float32)
    Wr = np.asarray(Wr, dtype=np.float32)
    Wo = np.asarray(Wo, dtype=np.float32)
    u = np.asarray(u, dtype=np.float32)
    v = np.asarray(v, dtype=np.float32)
    gamma = np.asarray(gamma, dtype=np.float32)
    beta = np.asarray(beta, dtype=np.float32)

    if "nc" not in _CACHE:
        _CACHE["nc"] = _build()
    nc = _CACHE["nc"]

    in_maps = [
        _prep_core(c, query, key_value, relative, Wq, Wk, Wv, Wr, Wo, u, v,
                   gamma, beta)
        for c in range(8)
    ]
    import os
    trace = bool(int(os.environ.get("KERNEL_TRACE", "0")))
    kwargs = {}
    if trace:
        kwargs = {"trace": True, "trace_cores": [0]}
    res = run_bass_kernel_spmd(nc, in_maps, core_ids=list(range(8)), **kwargs)
    _CACHE["last_result"] = res

    out = np.empty((B, TQ, D), dtype=np.float32)
    for c in range(8):
        b, half = c // 2, c % 2
        o = res.results[c]["out"].reshape(512, 1024)
        rows = np.concatenate(
            [np.arange(128 * qi, 128 * qi + 128) for qi in QSLOTS[half]])
        out[b][rows] = o
    return out
